# revision 8
# baseline (speedup 1.0000x reference)
"""Trainium2 Bass kernel for gated pair-bias attention (AlphaFold-style).

Reference computation (B=4, Q=K=2048, C=512, H=8, D=64):
    q = (q_x @ Wq^T)/sqrt(D); k = kv_x @ Wk^T; v = kv_x @ Wv^T      [B,H,S,D]
    a = softmax(q k^T + bias_mask + bias_pair)                       [B,H,Q,K]
    o = (a @ v) * sigmoid(q_x @ Wg^T + bg)                           [B,Q,H*D]
    out = o @ Wo^T + bo                                              [B,Q,C]

Sharding: one head per NeuronCore (8 heads = 8 cores), each core handling all
4 batches for its head.  This minimizes bias_pair traffic (each 16.8MB head
slice is loaded by exactly one core and reused across the 4 batches).  The
output projection is head-partial: out = sum_h og_h @ Wo_h^T, so each core
returns a partial [B,Q,C] and the host sums the 8 partials (+bo).

On-chip layouts (all transposed so the contraction dim is the partition dim):
    qT,kT [D=64, S]   from packed projections (q rows 0-63 / g rows 64-127,
                      k rows 0-63 / v rows 64-127) via host-stacked weights
    scores^T [k, q]   = kT_slice.T @ qT  -> softmax along PARTITION dim k:
                      no max-subtraction (logits bounded ~±9), the k-sum
                      comes free from a ones-column appended to V.
    exp(pair) is precomputed on host: exp(s+m+p) = exp(s+m)*exp(p), so the
    pair merge is a cheap SBUF*SBUF multiply instead of a PSUM-read add.
    o^T [65, q] accumulates in PSUM over 16 k-chunks (row 64 = softmax denom).
Matmuls run as float32r (TF32-like: fp32 storage, full PE rate at N>=512).
"""

import sys

sys.path.insert(0, "/opt/trn_rl_repo")

import numpy as np

import concourse.bass as bass
import concourse.bacc as bacc
import concourse.tile as tile
from concourse import mybir
from concourse.masks import make_identity

F32 = mybir.dt.float32
F32R = mybir.dt.float32r

# Problem constants (hardcoded per the harness contract)
B, S, C, H, D = 4, 2048, 512, 8, 64
NCORES = 8
QS = 512          # q-slice width (max fp32 moving operand)
P = 128           # partitions / k-chunk size
NCC = C // P      # contraction chunks for projections (4)


def build_nc(nb=B, s=S):
    """Build the per-core Bass program. nb/s shrinkable for simulation."""
    nq = s // QS          # q-slices
    nk = s // P           # k-chunks
    nss = s // QS         # projection s-slices

    nc = bacc.Bacc(None)

    xqT = nc.declare_dram_parameter("xqT", [nb, C, s], F32R, isOutput=False)
    xkT = nc.declare_dram_parameter("xkT", [nb, C, s], F32R, isOutput=False)
    pairE = nc.declare_dram_parameter("pairE", [s, s], F32, isOutput=False)
    maskr = nc.declare_dram_parameter("maskr", [nb, s], F32, isOutput=False)
    wqg = nc.declare_dram_parameter("wqg", [C, P], F32R, isOutput=False)
    wkv = nc.declare_dram_parameter("wkv", [C, P], F32R, isOutput=False)
    wo = nc.declare_dram_parameter("wo", [D, C], F32R, isOutput=False)
    bg = nc.declare_dram_parameter("bg", [D, 1], F32, isOutput=False)
    out = nc.declare_dram_parameter("out", [nb, s, C], F32, isOutput=True)

    with tile.TileContext(nc) as tc:
        with (
            tc.tile_pool(name="consts", bufs=1) as consts,
            tc.tile_pool(name="persist", bufs=1) as persist,
            tc.tile_pool(name="stream", bufs=4) as stream,
            tc.tile_pool(name="ptp", bufs=3) as ptp,
            tc.tile_pool(name="p2p", bufs=3) as p2p,
            tc.tile_pool(name="epi", bufs=2) as epi,
            tc.tile_pool(name="outp", bufs=3) as outp,
            tc.tile_pool(name="ps", bufs=4, space="PSUM") as psp,
            tc.tile_pool(name="oacc", bufs=4, space="PSUM") as oaccp,
        ):
            # ---- constants ----
            wqg_sb = consts.tile([P, NCC, P], F32R)
            nc.sync.dma_start(out=wqg_sb, in_=wqg[:, :].rearrange("(g p) m -> p g m", p=P))
            wkv_sb = consts.tile([P, NCC, P], F32R)
            nc.sync.dma_start(out=wkv_sb, in_=wkv[:, :].rearrange("(g p) m -> p g m", p=P))
            wo_sb = consts.tile([P, C], F32R)          # rows 64-127 hold Wo_h^T
            nc.sync.dma_start(out=wo_sb[D:P, :], in_=wo[:, :])
            bgv = consts.tile([P, 1], F32)
            nc.sync.dma_start(out=bgv[D:P, :], in_=bg[:, :])
            mask_sb = consts.tile([P, nb, nk], F32)
            nc.sync.dma_start(out=mask_sb, in_=maskr[:, :].rearrange("b (kc p) -> p b kc", p=P))
            ident32 = consts.tile([P, P], F32)
            make_identity(nc, ident32)
            ident = consts.tile([P, P], F32R)
            nc.vector.tensor_copy(out=ident, in_=ident32)
            ones32 = consts.tile([P, 1], F32)
            nc.vector.memset(ones32, 1.0)

            # ---- persistent per-batch tensors ----
            qgT = persist.tile([P, nb, s], F32R)   # rows 0-63 qT (pre-scaled), 64-127 sigmoid(g)T
            kvT = persist.tile([P, nb, s], F32R)   # rows 0-63 kT, 64-127 vT
            vaug = persist.tile([P, nb, nk, D + 1], F32R)  # V chunks + ones col
            nc.vector.tensor_copy(
                out=vaug[:, :, :, D : D + 1],
                in_=bass.AP(
                    tensor=ones32.tensor,
                    offset=ones32.offset,
                    ap=[ones32.ap[0], [0, nb], [0, nk], [0, 1]],
                ),
            )

            # ================= Phase A: projections =================
            for b in range(nb):
                for ss in range(nss):
                    sl = slice(ss * QS, (ss + 1) * QS)
                    xq_t = stream.tile([P, NCC, QS], F32R, tag="stream")
                    nc.sync.dma_start(
                        out=xq_t, in_=xqT[b, :, sl].rearrange("(g p) s -> p g s", p=P)
                    )
                    ps_qg = psp.tile([P, QS], F32, tag="ps")
                    for cc in range(NCC):
                        nc.tensor.matmul(
                            ps_qg,
                            lhsT=wqg_sb[:, cc, :],
                            rhs=xq_t[:, cc, :],
                            start=(cc == 0),
                            stop=(cc == NCC - 1),
                        )
                    nc.vector.tensor_copy(out=qgT[0:D, b, sl], in_=ps_qg[0:D, :])
                    nc.scalar.activation(
                        out=qgT[D:P, b, sl],
                        in_=ps_qg[D:P, :],
                        func=mybir.ActivationFunctionType.Sigmoid,
                        bias=bgv[D:P, :],
                    )

                    xk_t = stream.tile([P, NCC, QS], F32R, tag="stream")
                    nc.sync.dma_start(
                        out=xk_t, in_=xkT[b, :, sl].rearrange("(g p) s -> p g s", p=P)
                    )
                    ps_kv = psp.tile([P, QS], F32, tag="ps")
                    for cc in range(NCC):
                        nc.tensor.matmul(
                            ps_kv,
                            lhsT=wkv_sb[:, cc, :],
                            rhs=xk_t[:, cc, :],
                            start=(cc == 0),
                            stop=(cc == NCC - 1),
                        )
                    nc.vector.tensor_copy(out=kvT[:, b, sl], in_=ps_kv)

                    # V chunks for this s-slice: transpose vT [64,128] -> [128,64]
                    for j in range(QS // P):
                        kc = ss * (QS // P) + j
                        csl = slice(ss * QS + j * P, ss * QS + (j + 1) * P)
                        ps_t = psp.tile([P, QS], F32R, tag="ps")
                        nc.tensor.transpose(
                            out=ps_t[:, 0:D],
                            in_=kvT[D:P, b, csl],
                            identity=ident[D:P, D:P],
                        )
                        nc.vector.tensor_copy(out=vaug[:, b, kc, 0:D], in_=ps_t[:, 0:D])

            # ================= Phase B: attention =================
            for qs in range(nq):
                qsl = slice(qs * QS, (qs + 1) * QS)
                o_acc = [
                    oaccp.tile([D + 1, QS], F32, tag="oacc", name=f"oacc_q{qs}_b{bb}")
                    for bb in range(nb)
                ]
                for kc in range(nk):
                    if kc % 4 == 0:
                        kg = kc // 4
                        pair_t = stream.tile([P, 4, QS], F32, tag="stream")
                        nc.sync.dma_start(
                            out=pair_t,
                            in_=pairE[kg * 4 * P : (kg + 1) * 4 * P, qsl].rearrange(
                                "(g p) q -> p g q", p=P
                            ),
                        )
                    ksl = slice(kc * P, (kc + 1) * P)
                    for b in range(nb):
                        s_ps = psp.tile([P, QS], F32, tag="ps")
                        nc.tensor.matmul(
                            s_ps,
                            lhsT=kvT[0:D, b, ksl],
                            rhs=qgT[0:D, b, qsl],
                            start=True,
                            stop=True,
                        )
                        pt = ptp.tile([P, QS], F32, tag="pt")
                        nc.scalar.activation(
                            out=pt,
                            in_=s_ps,
                            func=mybir.ActivationFunctionType.Exp,
                            bias=mask_sb[:, b, kc : kc + 1],
                        )
                        p2 = p2p.tile([P, QS], F32R, tag="p2")
                        nc.vector.tensor_mul(out=p2, in0=pt, in1=pair_t[:, kc % 4, :])
                        nc.tensor.matmul(
                            o_acc[b],
                            lhsT=vaug[:, b, kc, :],
                            rhs=p2,
                            start=(kc == 0),
                            stop=(kc == nk - 1),
                        )
                # epilogue: gate, project, then normalize per-partition
                # (out_un = (o*g) @ Wo^T; out = out_un * recip(denom)[q] — the
                # softmax denom is per-q, which is the PARTITION dim after the
                # output projection, so normalization fuses into the PSUM
                # read-out as a tensor_scalar_mul.)
                for b in range(nb):
                    o_sb = epi.tile([D, QS], F32R, tag="osb0")
                    nc.vector.tensor_copy(out=o_sb, in_=o_acc[b][0:D, :])
                    og = epi.tile([P, QS], F32R, tag="og")
                    nc.sync.dma_start(out=og[D:P, :], in_=o_sb)  # shift to p64-127
                    nc.vector.tensor_mul(
                        out=og[D:P, :], in0=og[D:P, :], in1=qgT[D:P, b, qsl]
                    )
                    sums_sb = epi.tile([D + 1, QS], F32, tag="sums")
                    nc.vector.tensor_copy(
                        out=sums_sb[D : D + 1, :], in_=o_acc[b][D : D + 1, :]
                    )
                    for st in range(QS // P):
                        ssl = slice(st * P, (st + 1) * P)
                        trp = psp.tile([P, QS], F32, tag="ps")
                        nc.tensor.transpose(
                            out=trp[:, 0:1],
                            in_=sums_sb[D : D + 1, ssl],
                            identity=ident32[D : D + 1, D : D + 1],
                        )
                        rc = epi.tile([P, 1], F32, tag="rc")
                        nc.vector.reciprocal(out=rc, in_=trp[:, 0:1])
                        ops = psp.tile([P, QS], F32, tag="ps")
                        nc.tensor.matmul(
                            ops,
                            lhsT=og[D:P, ssl],
                            rhs=wo_sb[D:P, :],
                            start=True,
                            stop=True,
                            tile_position=(D, 0),
                        )
                        osb = outp.tile([P, C], F32, tag="osb")
                        nc.vector.tensor_scalar_mul(osb, ops, rc)
                        nc.sync.dma_start(
                            out=out[b, qs * QS + st * P : qs * QS + (st + 1) * P, :],
                            in_=osb,
                        )
    nc.compile()
    return nc


def prep_inputs(q_x, kv_x, bias_mask, bias_pair, Wq, Wk, Wv, Wo, bo, Wg, bg):
    """Host-side sharding/layout prep. Returns per-core input maps."""
    q_x = np.asarray(q_x, dtype=np.float32)
    kv_x = np.asarray(kv_x, dtype=np.float32)
    bias_mask = np.asarray(bias_mask, dtype=np.float32)
    bias_pair = np.asarray(bias_pair, dtype=np.float32)
    Wq = np.asarray(Wq, dtype=np.float32)
    Wk = np.asarray(Wk, dtype=np.float32)
    Wv = np.asarray(Wv, dtype=np.float32)
    Wo = np.asarray(Wo, dtype=np.float32)
    Wg = np.asarray(Wg, dtype=np.float32)
    bg = np.asarray(bg, dtype=np.float32)

    xqT = np.ascontiguousarray(q_x.transpose(0, 2, 1))
    xkT = np.ascontiguousarray(kv_x.transpose(0, 2, 1))
    maskr = np.ascontiguousarray(bias_mask[:, 0, 0, :])
    scale = 1.0 / np.sqrt(D)

    in_maps = []
    for h in range(NCORES):
        hs = slice(h * D, (h + 1) * D)
        wqg_h = np.concatenate([Wq[hs].T * scale, Wg[hs].T], axis=1)  # [C,128]
        wkv_h = np.concatenate([Wk[hs].T, Wv[hs].T], axis=1)          # [C,128]
        wo_h = np.ascontiguousarray(Wo[:, hs].T)                      # [64,C]
        pairE_h = np.exp(bias_pair[0, h].T)                           # [K,Q]
        in_maps.append(
            {
                "xqT": xqT,
                "xkT": xkT,
                "pairE": np.ascontiguousarray(pairE_h, dtype=np.float32),
                "maskr": maskr,
                "wqg": np.ascontiguousarray(wqg_h, dtype=np.float32),
                "wkv": np.ascontiguousarray(wkv_h, dtype=np.float32),
                "wo": wo_h,
                "bg": np.ascontiguousarray(bg[hs].reshape(D, 1)),
            }
        )
    return in_maps


_NC_CACHE = {}


def run(inputs, trace=False):
    from concourse.bass_utils import run_bass_kernel_spmd

    if "nc" not in _NC_CACHE:
        _NC_CACHE["nc"] = build_nc()
    nc = _NC_CACHE["nc"]
    in_maps = prep_inputs(**inputs)
    res = run_bass_kernel_spmd(nc, in_maps, list(range(NCORES)), trace=trace)
    bo = np.asarray(inputs["bo"], dtype=np.float32)
    total = res.results[0]["out"].astype(np.float32)
    for i in range(1, NCORES):
        total = total + res.results[i]["out"].astype(np.float32)
    total = total + bo[None, None, :]
    return total, res


def kernel(**inputs):
    out, _ = run(inputs, trace=False)
    return out
